# revision 28
# baseline (speedup 1.0000x reference)
"""Causal attention with ALiBi for Trainium2, tensor-parallel over heads x
data-parallel over batch (8 NeuronCores).

Problem: B=4, S=2048, D=2048, NH=16, HD=128, fp32.
  q/k/v = x @ Wq/Wk/Wv ; scores = q k^T / sqrt(HD) + alibi ; causal softmax ;
  out = (probs @ v) @ Wo

Sharding: core (b, j) handles batch b and the 8 interleaved heads
  j, j+2, ..., j+14 (interleaving balances steep/shallow ALiBi slopes so the
  per-core block-skipping is symmetric).  Each core returns out_partial^T;
  the host sums the two per-batch partials and transposes back.

On-core pipeline (bf16 matmul operands everywhere, fp32 PSUM accumulation;
all inputs arrive host-pre-arranged so every DMA reads >=1KB-contiguous runs
per partition: x as x^T bf16 [128, dc, s], W's as [p, head, dc, f]).
Heads are processed shallowest-slope first so the attention-heavy heads run
while x is still streaming in; the two steep heads drain the tail.

Per head, a software-pipelined schedule per s-slice st (= q-tile qt):
  1. deferred diagonal blocks of q-tile st-1: their scores/shift/exp/mask
     chains are emitted first, and their PV matmuls land right after the Q
     projection matmuls below, by which time the chains have drained.
  2. Q^T[:, st] = Wq-chunks^T @ XT  (16 matmuls, PSUM, ACT copy to bf16)
  3. softmax sums + normalize of q-tile st-1 (one ones-column matmul turns
     the elementwise-accumulated exp tile into the softmax sums; reciprocal
     on DVE, gpsimd partition_broadcast, normalize-multiply on DVE).  One
     stage late so the PE never waits on the DVE accumulation chain.
  4. attention blocks of q-tile st whose K/V are already resident
     (kc < 4st), with the K^T and V matmuls of THIS s-slice interleaved
     between blocks as PE filler work, so the PE never waits on the ACT exp
     chain.  V is computed directly in [k, hd] layout with XT chunks as the
     stationary operand (no PE transposes anywhere in the kernel); its four
     PSUM accumulation groups alternate between two banks so a group's
     start never waits on the previous group's PSUM->SBUF copy.
  5. the 4 diagonal blocks' scores/exp chains (their K/V just landed); a
     gpsimd affine_select masks the partial causal band; their PVs defer
     to step 1 of the next stage.
Softmax: exp(scores*scale + alibi[k] + shift[q]); the per-q shift folds
into the ACT bias column per (head, q-tile) for shallow heads (exact
cancellation) and is applied per-block on DVE for the 2 steep heads.
Blocks whose ALiBi decay is < e^-9 of the softmax sum are skipped, and
within kept blocks the q-columns beyond the decay window are not computed
(bf16 matmuls run 1 cyc/row at any width, so narrow blocks are cheap).
O^T tiles for qt=0 stay in SBUF (bf16); qt>0 spill to DRAM per (h, q-tile).
out^T = Wo_j^T @ O^T accumulated over the 8 heads (bf16); Wo prefetches as
one contiguous tile during the last head so the output stage starts with no
bubble, and its PSUM tiles alternate between two rings so the output copies
are never on the critical path.
"""

import math

import numpy as np

B, S, D, NH = 4, 2048, 2048, 16
HD = D // NH            # 128
NHG = NH // 2           # heads per core
DC = D // 128           # 16 d-chunks
QT_TILES = S // 512     # 4 q tiles
SCALE = 1.0 / math.sqrt(HD)

_cache = {}


def _get_slopes(n):
    def pow2(n):
        start = 2 ** (-(2 ** (-(math.log2(n) - 3))))
        return [start * start**i for i in range(n)]

    if math.log2(n).is_integer():
        return pow2(n)
    c = 2 ** math.floor(math.log2(n))
    return pow2(c) + _get_slopes(2 * c)[0::2][: n - c]


def _build():
    import concourse.bacc as bacc
    import concourse.mybir as mybir
    import concourse.tile as tile
    from concourse.bass import ts

    f32 = mybir.dt.float32
    f32r = mybir.dt.float32r
    bf16 = mybir.dt.bfloat16
    Exp = mybir.ActivationFunctionType.Exp

    nc = bacc.Bacc()
    # x arrives pre-transposed (host-side) as bf16 [D, S]
    x_in = nc.declare_dram_parameter("x", [D, S], bf16, isOutput=False)
    # weights arrive pre-arranged by the host as [p, head, dc, f] so each
    # per-head DMA reads 4KB-contiguous runs per partition
    wq_in = nc.declare_dram_parameter("wq", [128, NHG, DC, HD], bf16,
                                      isOutput=False)
    wk_in = nc.declare_dram_parameter("wk", [128, NHG, DC, HD], bf16,
                                      isOutput=False)
    wv_in = nc.declare_dram_parameter("wv", [128, NHG, DC, HD], bf16,
                                      isOutput=False)
    wo_in = nc.declare_dram_parameter("wo", [128, NHG, D], bf16,
                                      isOutput=False)
    # alibi_b[p, ((h*16+kc)*4+qt)] = -slope_h*(S-1-(kc*128+p)) + C[h,qt]
    # C folds the per-q-tile softmax shift for heads with small slope.
    alibi_b_in = nc.declare_dram_parameter(
        "alibi_b", [128, NHG * DC * QT_TILES], f32, isOutput=False)
    # alibi_q[h, q] = +slope_h * (S-1 - q)   (per-query shift)
    alibi_q_in = nc.declare_dram_parameter("alibi_q", [NHG, S], f32,
                                           isOutput=False)
    ones_col_in = nc.declare_dram_parameter("ones_col", [128, 1], f32r,
                                            isOutput=False)
    outT = nc.declare_dram_parameter("outT", [D, S], f32, isOutput=True)

    ot_scratch = nc.dram_tensor("ot_scratch", [NHG, 128, S], bf16)

    # heads are interleaved across the two cores of a batch (core parity j
    # gets global heads j, j+2, ...).  Skip counts use the SHALLOWER
    # parity's slope so one SPMD program is valid for both.
    slope_c = [0.7071067811865476 ** (2 * hh + 2) for hh in range(NHG)]

    def n_skip(h, qt):
        # contribution of a skipped block is < e^-9 of the softmax sum
        dist = int(9.0 / slope_c[h]) + 1
        return max(0, (512 * qt - dist - 127) // 128 + 1)

    with tile.TileContext(nc) as tc:
        with (
            tc.tile_pool(name="consts", bufs=1) as pc,
            tc.tile_pool(name="oz", bufs=8) as po0,
            tc.tile_pool(name="wohi", bufs=1) as pwo_hi,
            tc.tile_pool(name="psA", bufs=2, space="PSUM") as psA,
            tc.tile_pool(name="psST", bufs=3, space="PSUM") as psST,
        ):
            alibi_sb = pc.tile([128, NHG * DC * QT_TILES], f32,
                               name="alibi_sb")
            ones_col = pc.tile([128, 1], f32r, name="ones_col_sb")

            ot0_tiles = {}       # per-head qt=0 O^T tiles, kept in SBUF
            wo_sb_box = []

            def load_wo():
                wo_sb = pwo_hi.tile([128, NHG, D], bf16, tag="wo",
                                    name="wo_sb")
                nc.sync.dma_start(wo_sb[:], wo_in[:])
                wo_sb_box.append(wo_sb)

            with (
                tc.tile_pool(name="xt", bufs=1) as pxt,
                tc.tile_pool(name="wp", bufs=2) as pw,
                tc.tile_pool(name="qkv2", bufs=2) as pq2,
                tc.tile_pool(name="qkv", bufs=2) as pq,
                tc.tile_pool(name="att", bufs=2) as pa,
                tc.tile_pool(name="epool", bufs=8) as pe_pool,
                tc.tile_pool(name="small", bufs=2) as psm,
            ):
                XT = pxt.tile([128, DC, S], bf16, name="XT")
                xt_view = x_in.rearrange("(dc p) s -> p dc s", p=128)

                # pending softmax-sum/normalize work, emitted one stage late:
                # (h, qt, pot, eacc)
                pending = []
                # pending diagonal attention blocks: (attn_block_fn, kcs)
                pending_diag = []

                def flush_diag():
                    if not pending_diag:
                        return
                    pre, pv, kcs = pending_diag.pop()
                    done = [(kc,) + pre(kc) for kc in kcs]
                    for kc, rw, e_sb in done:
                        pv(kc, rw, e_sb)

                def emit_norm():
                    if not pending:
                        return
                    h, qt, pot, eacc = pending.pop()
                    psums = psA.tile([1, 512], f32, tag="pot", name="psums")
                    nc.tensor.matmul(psums[:], ones_col[:], eacc[:],
                                     start=True, stop=True)
                    recip = psm.tile([1, 512], f32, tag="recip", name="recip")
                    nc.vector.reciprocal(recip[:], psums[:])
                    bc_sb = pa.tile([128, 512], f32, tag="bc", name="bc_sb")
                    nc.gpsimd.partition_broadcast(bc_sb[:], recip[:])
                    if qt == 0:
                        ot_sb = po0.tile([128, 512], bf16, tag="ot0",
                                         name="ot0_sb")
                        ot0_tiles[h] = ot_sb
                    else:
                        ot_sb = pa.tile([128, 512], bf16, tag="ot",
                                        name="ot_sb")
                    nc.vector.tensor_mul(out=ot_sb[:], in0=pot[:],
                                         in1=bc_sb[:])
                    if qt != 0:
                        nc.sync.dma_start(ot_scratch[h, :, ts(qt, 512)],
                                          ot_sb[:])

                def emit_head(h):
                    qt_sb = pq2.tile([128, S], bf16, tag="QT", name="qt_sb")
                    kt_sb = pq2.tile([128, S], bf16, tag="KT", name="kt_sb")
                    v_sb = pq.tile([128, DC, HD], bf16, tag="V", name="v_sb")
                    w_sbs = []
                    for w_in, wtag in ((wq_in, "wq"), (wk_in, "wk"),
                                       (wv_in, "wv")):
                        w_sb = pw.tile([128, DC, HD], bf16, tag=wtag,
                                       name="w_sb")
                        if h == NHG - 1:
                            # first head: first x quarter, then wq, then the
                            # remaining quarters; wk/wv follow (needed only
                            # once the K/V fillers start)
                            if wtag == "wq":
                                nc.sync.dma_start(
                                    XT[:, ts(0, 2), ts(0, 512)],
                                    xt_view[:, ts(0, 2), ts(0, 512)])
                                nc.scalar.dma_start(w_sb[:], w_in[:, h])
                                nc.sync.dma_start(
                                    XT[:, ts(1, 2), ts(0, 512)],
                                    xt_view[:, ts(1, 2), ts(0, 512)])
                                for qi in range(1, 4):
                                    nc.sync.dma_start(
                                        XT[:, ts(qi, 4), ts(0, 512)],
                                        xt_view[:, ts(qi, 4), ts(0, 512)])
                            elif wtag == "wk":
                                nc.sync.dma_start(alibi_sb[:], alibi_b_in[:])
                                nc.sync.dma_start(ones_col[:],
                                                  ones_col_in[:])
                                nc.scalar.dma_start(w_sb[:], w_in[:, h])
                            else:
                                nc.scalar.dma_start(w_sb[:], w_in[:, h])
                        else:
                            nc.sync.dma_start(w_sb[:], w_in[:, h])
                        w_sbs.append(w_sb)
                    if h == NHG - 1:
                        for st in range(1, QT_TILES):
                            nc.sync.dma_start(XT[:, :, ts(st, 512)],
                                              xt_view[:, :, ts(st, 512)])
                    if h == 0:
                        load_wo()

                    steep = h < 2
                    for st in range(QT_TILES):
                        qt = st
                        nkc = 4 * (qt + 1)
                        kc0 = n_skip(h, qt)
                        shift_sb = None
                        if steep:
                            shift_sb = psm.tile([128, 512], f32, tag="shift",
                                                name="shift_sb")
                            nc.sync.dma_start(
                                shift_sb[:],
                                alibi_q_in[h, ts(qt, 512)]
                                .partition_broadcast(128))

                        # previous q-tile's diagonal blocks: emit their
                        # scores/exp chains first, then interleave their PV
                        # matmuls into the Q projection so the PE never waits
                        # on the 4-hop scores->shift->exp->mask latency
                        diag_pvs = []
                        if pending_diag:
                            pre, pv, kcs = pending_diag.pop()
                            for kc in kcs:
                                diag_pvs.append((pv, kc) + pre(kc))

                        # ---- Q projection for this s-slice ----
                        pp_q = psA.tile([128, 512], f32, tag="pp", bufs=3, name="pp_q")
                        for dc in range(DC):
                            nc.tensor.matmul(
                                pp_q[:], w_sbs[0][:, dc, :],
                                XT[:, dc, ts(st, 512)],
                                start=(dc == 0), stop=(dc == DC - 1),
                                skip_group_check=True)
                        for pv, kc, rw, e_sb in diag_pvs:
                            pv(kc, rw, e_sb)
                        nc.scalar.copy(qt_sb[:, ts(st, 512)], pp_q[:])

                        # ---- filler stream: K^T and V matmuls of this
                        # s-slice, interleaved between attention blocks so
                        # the PE never drains while ACT runs the exp chain.
                        pp_k = psA.tile([128, 512], f32, tag="pp", bufs=3, name="pp_k")
                        pp_v = psA.tile([128, 512], f32, tag="pp", bufs=3, name="pp_v")
                        pp_v2 = psA.tile([128, 512], f32, tag="pp", bufs=3,
                                         name="pp_v2")

                        def fill_units():
                            for dc in range(DC):
                                yield ("K", dc)
                            for j in range(4):
                                for dc in range(DC):
                                    yield ("V", j, dc)

                        filler = fill_units()

                        def take_fillers(n):
                            for _ in range(n):
                                u = next(filler, None)
                                if u is None:
                                    return
                                if u[0] == "K":
                                    dc = u[1]
                                    nc.tensor.matmul(
                                        pp_k[:], w_sbs[1][:, dc, :],
                                        XT[:, dc, ts(st, 512)],
                                        start=(dc == 0), stop=(dc == DC - 1),
                                        skip_group_check=True)
                                    if dc == DC - 1:
                                        nc.scalar.copy(
                                            kt_sb[:, ts(st, 512)], pp_k[:])
                                else:
                                    _, j, dc = u
                                    sc = st * 4 + j
                                    pv_t = pp_v if j % 2 == 0 else pp_v2
                                    nc.tensor.matmul(
                                        pv_t[:, ts(j, 128)],
                                        XT[:, dc, ts(sc, 128)],
                                        w_sbs[2][:, dc, :],
                                        start=(dc == 0), stop=(dc == DC - 1),
                                        skip_group_check=True)
                                    if dc == DC - 1:
                                        nc.scalar.copy(
                                            v_sb[:, sc, :],
                                            pv_t[:, ts(j, 128)])

                        pot = psA.tile([128, 512], f32, tag="pot", name="pot")
                        eacc = pa.tile([128, 512], f32r, tag="eacc",
                                       name="eacc")

                        dist_c = int(9.0 / slope_c[h]) + 1

                        def attn_pre(kc, qt=qt, kc0=kc0, eacc=eacc,
                                     shift_sb=shift_sb, dist_c=dist_c):
                            # columns < r are fully masked; columns >= w are
                            # below the ALiBi decay threshold for every key
                            # in this block (bf16 matmuls run 1 cyc/row at
                            # any width)
                            r = max(0, 128 * kc - 512 * qt)
                            w = min(512, 128 * (kc + 1) + dist_c - 512 * qt)
                            pst = psST.tile([128, 512], f32, tag="pst",
                                            name="pst")
                            nc.tensor.matmul(pst[:, r:w],
                                             kt_sb[:, ts(kc, 128)],
                                             qt_sb[:, 512 * qt + r:
                                                   512 * qt + w],
                                             start=True, stop=True)
                            e_sb = pe_pool.tile([128, 512], bf16, tag="e",
                                                name="e_sb")
                            col = (h * DC + kc) * QT_TILES + qt
                            if steep:
                                t1 = pa.tile([128, 512], f32, tag="t1",
                                             name="t1")
                                nc.vector.scalar_tensor_tensor(
                                    t1[:, r:w], pst[:, r:w], SCALE,
                                    shift_sb[:, r:w],
                                    mybir.AluOpType.mult,
                                    mybir.AluOpType.add)
                                nc.scalar.activation(
                                    e_sb[:, r:w], t1[:, r:w], Exp,
                                    bias=alibi_sb[:, col:col + 1],
                                    scale=1.0)
                            else:
                                nc.scalar.activation(
                                    e_sb[:, r:w], pst[:, r:w], Exp,
                                    bias=alibi_sb[:, col:col + 1],
                                    scale=SCALE)
                            if kc >= 4 * qt:
                                # keep where qf - kp - r >= 0 (k <= q)
                                nc.gpsimd.affine_select(
                                    e_sb[:, r:r + 128],
                                    e_sb[:, r:r + 128],
                                    pattern=[[1, 128]],
                                    compare_op=mybir.AluOpType.is_ge,
                                    fill=0.0,
                                    base=0,
                                    channel_multiplier=-1)
                            # accumulate exp blocks for the softmax sums
                            if kc == kc0:
                                if w < 512:
                                    nc.vector.memset(
                                        eacc[:, w:].bitcast(f32), 0.0)
                                nc.vector.tensor_copy(eacc[:, r:w],
                                                      e_sb[:, r:w])
                            else:
                                nc.vector.tensor_add(eacc[:, r:w],
                                                     eacc[:, r:w],
                                                     e_sb[:, r:w])
                            return (r, w), e_sb

                        def attn_pv(kc, rw, e_sb, kc0=kc0, nkc=nkc, pot=pot,
                                    v_sb=v_sb):
                            r, w = rw
                            nc.tensor.matmul(pot[:, r:w], v_sb[:, kc, :],
                                             e_sb[:, r:w],
                                             start=(kc == kc0),
                                             stop=(kc == nkc - 1))

                        def attn_block(kc):
                            rw, e_sb = attn_pre(kc)
                            attn_pv(kc, rw, e_sb)

                        take_fillers(10)
                        # softmax sums + normalize of the PREVIOUS q-tile
                        # (here its DVE accumulation chain has fully drained)
                        emit_norm()
                        # blocks whose K/V are resident from earlier s-slices
                        for kc in range(kc0, 4 * qt):
                            attn_block(kc)
                            take_fillers(2)
                        take_fillers(DC + 4 * DC)   # drain the rest
                        # diagonal blocks (K/V of this s-slice just landed)
                        # run at the start of the NEXT stage, where the next
                        # Q projection hides their exp latency
                        pending_diag.append(
                            (attn_pre, attn_pv, list(range(4 * qt, nkc))))
                        pending.append((h, qt, pot, eacc))

                for h in range(NHG - 1, -1, -1):
                    emit_head(h)
                flush_diag()
                emit_norm()

            # ---- out^T = Wo_g^T @ O^T (XT pool closed) ----
            with (
                tc.tile_pool(name="otl", bufs=2) as pot_l,
                tc.tile_pool(name="ost", bufs=4) as post,
            ):
                wo_sb = wo_sb_box[0]
                for st in range(QT_TILES):
                    if st == 0:
                        ot_of = lambda h: ot0_tiles[h][:]
                    else:
                        ot_all = pot_l.tile([128, NHG, 512], bf16,
                                            tag="ot_all", name="ot_all")
                        for h in range(NHG):
                            nc.sync.dma_start(ot_all[:, h, :],
                                              ot_scratch[h, :, ts(st, 512)])
                        ot_of = lambda h, _t=ot_all: _t[:, h, :]
                    for mt in range(D // 128):
                        pp = psA.tile([128, 512], f32,
                                      tag="pp" if mt % 2 == 0 else "pot",
                                      bufs=3 if mt % 2 == 0 else None,
                                      name="pp")
                        for h in range(NHG):
                            nc.tensor.matmul(
                                pp[:],
                                wo_sb[:, h, ts(mt, 128)],
                                ot_of(h),
                                start=(h == 0), stop=(h == NHG - 1))
                        o_sb = post.tile([128, 512], f32, tag="osb",
                                         name="o_sb")
                        if st == QT_TILES - 1 and mt == D // 128 - 1:
                            for hf in range(2):
                                nc.scalar.copy(o_sb[:, ts(hf, 256)],
                                               pp[:, ts(hf, 256)])
                                nc.sync.dma_start(
                                    outT[ts(mt, 128),
                                         512 * st + 256 * hf:
                                         512 * st + 256 * (hf + 1)],
                                    o_sb[:, ts(hf, 256)])
                        else:
                            nc.scalar.copy(o_sb[:], pp[:])
                            nc.sync.dma_start(outT[ts(mt, 128), ts(st, 512)],
                                              o_sb[:])

    nc.compile()
    return nc


def _in_maps(x, Wq, Wk, Wv, Wo):
    import ml_dtypes

    bf16 = ml_dtypes.bfloat16
    slopes = np.asarray(_get_slopes(NH), dtype=np.float32)
    pos = np.arange(S, dtype=np.float32)
    dist = np.float32(S - 1) - pos                       # (S,)
    ones_col = np.ones((128, 1), np.float32)

    in_maps = []
    for b in range(B):
        xT = np.ascontiguousarray(x[b].T).astype(bf16)
        for g in range(2):
            heads = list(range(g, NH, 2))                 # interleaved
            sl = slopes[heads]                            # (8,)
            # alibi_b[p, ((h*DC+kc)*QT+qt)] = -sl[h]*dist[kc*128+p] + C[h,qt]
            ab = np.empty((128, NHG * DC * QT_TILES), np.float32)
            d2 = dist.reshape(DC, 128)                    # [kc, p]
            for h in range(NHG):
                for kc in range(DC):
                    a_col = (-sl[h] * d2[kc]).astype(np.float32)  # (128,)
                    for qt in range(QT_TILES):
                        if h < 2:
                            c = np.float32(0.0)
                        else:
                            q_mid = 512 * qt + 255.5
                            c = np.float32(sl[h] * (S - 1 - q_mid))
                        ab[:, (h * DC + kc) * QT_TILES + qt] = a_col + c
            alibi_q = (sl[:, None] * dist[None, :]).astype(np.float32)
            def w_fwd(W):
                # [D, NHG*HD] head-sliced -> [p, h, dc, f]
                arr = np.stack([W[:, h * HD:(h + 1) * HD] for h in heads],
                               axis=1)              # [D, NHG, HD]
                arr = arr.reshape(DC, 128, NHG, HD).transpose(1, 2, 0, 3)
                return np.ascontiguousarray(arr).astype(bf16)

            wo_arr = np.stack([Wo[h * HD:(h + 1) * HD, :] for h in heads],
                              axis=0)               # [NHG, HD, D]
            wo_arr = np.ascontiguousarray(
                wo_arr.transpose(1, 0, 2)).astype(bf16)   # [p, NHG, D]
            in_maps.append({
                "x": xT,
                "wq": w_fwd(Wq),
                "wk": w_fwd(Wk),
                "wv": w_fwd(Wv),
                "wo": wo_arr,
                "alibi_b": ab,
                "alibi_q": alibi_q,
                "ones_col": ones_col,
            })
    return in_maps


def kernel(x, Wq, Wk, Wv, Wo, _trace=False):
    from concourse.bass_utils import run_bass_kernel_spmd

    if "nc" not in _cache:
        _cache["nc"] = _build()
    nc = _cache["nc"]

    res = run_bass_kernel_spmd(
        nc, _in_maps(x, Wq, Wk, Wv, Wo), core_ids=list(range(2 * B)),
        trace=_trace)
    _cache["last_exec_time_ns"] = res.exec_time_ns

    out = np.empty((B, S, D), dtype=np.float32)
    for b in range(B):
        out[b] = (res.results[2 * b]["outT"] + res.results[2 * b + 1]["outT"]).T
    return out


# revision 29
# speedup vs baseline: 1.0005x; 1.0005x over previous
"""Causal attention with ALiBi for Trainium2, tensor-parallel over heads x
data-parallel over batch (8 NeuronCores).

Problem: B=4, S=2048, D=2048, NH=16, HD=128, fp32.
  q/k/v = x @ Wq/Wk/Wv ; scores = q k^T / sqrt(HD) + alibi ; causal softmax ;
  out = (probs @ v) @ Wo

Sharding: core (b, j) handles batch b and the 8 interleaved heads
  j, j+2, ..., j+14 (interleaving balances steep/shallow ALiBi slopes so the
  per-core block-skipping is symmetric).  Each core returns out_partial^T;
  the host sums the two per-batch partials and transposes back.

On-core pipeline (bf16 matmul operands everywhere, fp32 PSUM accumulation;
all inputs arrive host-pre-arranged so every DMA reads >=1KB-contiguous runs
per partition: x as x^T bf16 [128, dc, s], W's as [p, head, dc, f]).
Heads are processed shallowest-slope first so the attention-heavy heads run
while x is still streaming in; the two steep heads drain the tail.

Per head, a software-pipelined schedule per s-slice st (= q-tile qt):
  1. deferred diagonal blocks of q-tile st-1: their scores/shift/exp/mask
     chains are emitted first, and their PV matmuls land right after the Q
     projection matmuls below, by which time the chains have drained.
  2. Q^T[:, st] = Wq-chunks^T @ XT  (16 matmuls, PSUM, ACT copy to bf16)
  3. softmax sums + normalize of q-tile st-1 (one ones-column matmul turns
     the elementwise-accumulated exp tile into the softmax sums; reciprocal
     on DVE, gpsimd partition_broadcast, normalize-multiply on DVE).  One
     stage late so the PE never waits on the DVE accumulation chain.
  4. attention blocks of q-tile st whose K/V are already resident
     (kc < 4st), with the K^T and V matmuls of THIS s-slice interleaved
     between blocks as PE filler work, so the PE never waits on the ACT exp
     chain.  V is computed directly in [k, hd] layout with XT chunks as the
     stationary operand (no PE transposes anywhere in the kernel); its four
     PSUM accumulation groups alternate between two banks so a group's
     start never waits on the previous group's PSUM->SBUF copy.
  5. the 4 diagonal blocks' scores/exp chains (their K/V just landed); a
     gpsimd affine_select masks the partial causal band; their PVs defer
     to step 1 of the next stage.
Softmax: exp(scores*scale + alibi[k] + shift[q]); the per-q shift folds
into the ACT bias column per (head, q-tile) for shallow heads (exact
cancellation) and is applied per-block on DVE for the 2 steep heads.
Blocks whose ALiBi decay is < e^-9 of the softmax sum are skipped, and
within kept blocks the q-columns beyond the decay window are not computed
(bf16 matmuls run 1 cyc/row at any width, so narrow blocks are cheap).
O^T tiles for qt=0 stay in SBUF (bf16); qt>0 spill to DRAM per (h, q-tile).
out^T = Wo_j^T @ O^T accumulated over the 8 heads (bf16); Wo prefetches as
one contiguous tile during the last head so the output stage starts with no
bubble, and its PSUM tiles alternate between two rings so the output copies
are never on the critical path.
"""

import math

import numpy as np

B, S, D, NH = 4, 2048, 2048, 16
HD = D // NH            # 128
NHG = NH // 2           # heads per core
DC = D // 128           # 16 d-chunks
QT_TILES = S // 512     # 4 q tiles
SCALE = 1.0 / math.sqrt(HD)

_cache = {}


def _get_slopes(n):
    def pow2(n):
        start = 2 ** (-(2 ** (-(math.log2(n) - 3))))
        return [start * start**i for i in range(n)]

    if math.log2(n).is_integer():
        return pow2(n)
    c = 2 ** math.floor(math.log2(n))
    return pow2(c) + _get_slopes(2 * c)[0::2][: n - c]


def _build():
    import concourse.bacc as bacc
    import concourse.mybir as mybir
    import concourse.tile as tile
    from concourse.bass import ts

    f32 = mybir.dt.float32
    f32r = mybir.dt.float32r
    bf16 = mybir.dt.bfloat16
    Exp = mybir.ActivationFunctionType.Exp

    nc = bacc.Bacc()
    # x arrives pre-transposed (host-side) as bf16 [D, S]
    x_in = nc.declare_dram_parameter("x", [D, S], bf16, isOutput=False)
    # weights arrive pre-arranged by the host as [p, head, dc, f] so each
    # per-head DMA reads 4KB-contiguous runs per partition
    wq_in = nc.declare_dram_parameter("wq", [128, NHG, DC, HD], bf16,
                                      isOutput=False)
    wk_in = nc.declare_dram_parameter("wk", [128, NHG, DC, HD], bf16,
                                      isOutput=False)
    wv_in = nc.declare_dram_parameter("wv", [128, NHG, DC, HD], bf16,
                                      isOutput=False)
    wo_in = nc.declare_dram_parameter("wo", [128, NHG, D], bf16,
                                      isOutput=False)
    # alibi_b[p, ((h*16+kc)*4+qt)] = -slope_h*(S-1-(kc*128+p)) + C[h,qt]
    # C folds the per-q-tile softmax shift for heads with small slope.
    alibi_b_in = nc.declare_dram_parameter(
        "alibi_b", [128, NHG * DC * QT_TILES], f32, isOutput=False)
    # alibi_q[h, q] = +slope_h * (S-1 - q)   (per-query shift)
    alibi_q_in = nc.declare_dram_parameter("alibi_q", [NHG, S], f32,
                                           isOutput=False)
    ones_col_in = nc.declare_dram_parameter("ones_col", [128, 1], f32r,
                                            isOutput=False)
    outT = nc.declare_dram_parameter("outT", [D, S], f32, isOutput=True)

    ot_scratch = nc.dram_tensor("ot_scratch", [NHG, 128, S], bf16)

    # heads are interleaved across the two cores of a batch (core parity j
    # gets global heads j, j+2, ...).  Skip counts use the SHALLOWER
    # parity's slope so one SPMD program is valid for both.
    slope_c = [0.7071067811865476 ** (2 * hh + 2) for hh in range(NHG)]

    def n_skip(h, qt):
        # contribution of a skipped block is < e^-9 of the softmax sum
        dist = int(9.0 / slope_c[h]) + 1
        return max(0, (512 * qt - dist - 127) // 128 + 1)

    with tile.TileContext(nc) as tc:
        with (
            tc.tile_pool(name="consts", bufs=1) as pc,
            tc.tile_pool(name="oz", bufs=8) as po0,
            tc.tile_pool(name="wohi", bufs=1) as pwo_hi,
            tc.tile_pool(name="psA", bufs=2, space="PSUM") as psA,
            tc.tile_pool(name="psST", bufs=3, space="PSUM") as psST,
        ):
            alibi_sb = pc.tile([128, NHG * DC * QT_TILES], f32,
                               name="alibi_sb")
            ones_col = pc.tile([128, 1], f32r, name="ones_col_sb")

            ot0_tiles = {}       # per-head qt=0 O^T tiles, kept in SBUF
            wo_sb_box = []

            def load_wo():
                wo_sb = pwo_hi.tile([128, NHG, D], bf16, tag="wo",
                                    name="wo_sb")
                nc.sync.dma_start(wo_sb[:], wo_in[:])
                wo_sb_box.append(wo_sb)

            with (
                tc.tile_pool(name="xt", bufs=1) as pxt,
                tc.tile_pool(name="wp", bufs=2) as pw,
                tc.tile_pool(name="qkv2", bufs=2) as pq2,
                tc.tile_pool(name="qkv", bufs=2) as pq,
                tc.tile_pool(name="att", bufs=2) as pa,
                tc.tile_pool(name="epool", bufs=8) as pe_pool,
                tc.tile_pool(name="small", bufs=2) as psm,
            ):
                XT = pxt.tile([128, DC, S], bf16, name="XT")
                xt_view = x_in.rearrange("(dc p) s -> p dc s", p=128)

                # pending softmax-sum/normalize work, emitted one stage late:
                # (h, qt, pot, eacc)
                pending = []
                # pending diagonal attention blocks: (attn_block_fn, kcs)
                pending_diag = []

                def flush_diag():
                    if not pending_diag:
                        return
                    pre, pv, kcs = pending_diag.pop()
                    done = [(kc,) + pre(kc) for kc in kcs]
                    for kc, rw, e_sb in done:
                        pv(kc, rw, e_sb)

                def emit_norm():
                    if not pending:
                        return
                    h, qt, pot, eacc = pending.pop()
                    psums = psA.tile([1, 512], f32, tag="pot", name="psums")
                    nc.tensor.matmul(psums[:], ones_col[:], eacc[:],
                                     start=True, stop=True)
                    recip = psm.tile([1, 512], f32, tag="recip", name="recip")
                    nc.vector.reciprocal(recip[:], psums[:])
                    bc_sb = pa.tile([128, 512], f32, tag="bc", name="bc_sb")
                    nc.gpsimd.partition_broadcast(bc_sb[:], recip[:])
                    if qt == 0:
                        ot_sb = po0.tile([128, 512], bf16, tag="ot0",
                                         name="ot0_sb")
                        ot0_tiles[h] = ot_sb
                    else:
                        ot_sb = pa.tile([128, 512], bf16, tag="ot",
                                        name="ot_sb")
                    nc.vector.tensor_mul(out=ot_sb[:], in0=pot[:],
                                         in1=bc_sb[:])
                    if qt != 0:
                        nc.sync.dma_start(ot_scratch[h, :, ts(qt, 512)],
                                          ot_sb[:])

                def emit_head(h):
                    qt_sb = pq2.tile([128, S], bf16, tag="QT", name="qt_sb")
                    kt_sb = pq2.tile([128, S], bf16, tag="KT", name="kt_sb")
                    v_sb = pq.tile([128, DC, HD], bf16, tag="V", name="v_sb")
                    w_sbs = []
                    for w_in, wtag in ((wq_in, "wq"), (wk_in, "wk"),
                                       (wv_in, "wv")):
                        w_sb = pw.tile([128, DC, HD], bf16, tag=wtag,
                                       name="w_sb")
                        if h == NHG - 1:
                            # first head: first x quarter, then wq, then the
                            # remaining quarters; wk/wv follow (needed only
                            # once the K/V fillers start)
                            if wtag == "wq":
                                nc.sync.dma_start(
                                    XT[:, ts(0, 2), ts(0, 512)],
                                    xt_view[:, ts(0, 2), ts(0, 512)])
                                nc.scalar.dma_start(w_sb[:], w_in[:, h])
                                nc.sync.dma_start(
                                    XT[:, ts(1, 2), ts(0, 512)],
                                    xt_view[:, ts(1, 2), ts(0, 512)])
                                for qi in range(1, 4):
                                    nc.sync.dma_start(
                                        XT[:, ts(qi, 4), ts(0, 512)],
                                        xt_view[:, ts(qi, 4), ts(0, 512)])
                            elif wtag == "wk":
                                nc.sync.dma_start(alibi_sb[:], alibi_b_in[:])
                                nc.sync.dma_start(ones_col[:],
                                                  ones_col_in[:])
                                nc.scalar.dma_start(w_sb[:], w_in[:, h])
                            else:
                                nc.scalar.dma_start(w_sb[:], w_in[:, h])
                        else:
                            nc.sync.dma_start(w_sb[:], w_in[:, h])
                        w_sbs.append(w_sb)
                    if h == NHG - 1:
                        for st in range(1, QT_TILES):
                            nc.sync.dma_start(XT[:, :, ts(st, 512)],
                                              xt_view[:, :, ts(st, 512)])
                    if h == 0:
                        load_wo()

                    steep = h < 2
                    for st in range(QT_TILES):
                        qt = st
                        nkc = 4 * (qt + 1)
                        kc0 = n_skip(h, qt)
                        shift_sb = None
                        if steep:
                            shift_sb = psm.tile([128, 512], f32, tag="shift",
                                                name="shift_sb")
                            nc.sync.dma_start(
                                shift_sb[:],
                                alibi_q_in[h, ts(qt, 512)]
                                .partition_broadcast(128))

                        # previous q-tile's diagonal blocks: emit their
                        # scores/exp chains first, then interleave their PV
                        # matmuls into the Q projection so the PE never waits
                        # on the 4-hop scores->shift->exp->mask latency
                        diag_pvs = []
                        if pending_diag:
                            pre, pv, kcs = pending_diag.pop()
                            for kc in kcs:
                                diag_pvs.append((pv, kc) + pre(kc))

                        # ---- Q projection for this s-slice ----
                        pp_q = psA.tile([128, 512], f32, tag="pp", bufs=3, name="pp_q")
                        for dc in range(DC):
                            nc.tensor.matmul(
                                pp_q[:], w_sbs[0][:, dc, :],
                                XT[:, dc, ts(st, 512)],
                                start=(dc == 0), stop=(dc == DC - 1),
                                skip_group_check=True)
                        for pv, kc, rw, e_sb in diag_pvs:
                            pv(kc, rw, e_sb)
                        nc.scalar.copy(qt_sb[:, ts(st, 512)], pp_q[:])

                        # ---- filler stream: K^T and V matmuls of this
                        # s-slice, interleaved between attention blocks so
                        # the PE never drains while ACT runs the exp chain.
                        pp_k = psA.tile([128, 512], f32, tag="pp", bufs=3, name="pp_k")
                        pp_v = psA.tile([128, 512], f32, tag="pp", bufs=3, name="pp_v")
                        pp_v2 = psA.tile([128, 512], f32, tag="pp", bufs=3,
                                         name="pp_v2")

                        def fill_units():
                            for dc in range(DC):
                                yield ("K", dc)
                            for j in range(4):
                                for dc in range(DC):
                                    yield ("V", j, dc)

                        filler = fill_units()

                        def take_fillers(n):
                            for _ in range(n):
                                u = next(filler, None)
                                if u is None:
                                    return
                                if u[0] == "K":
                                    dc = u[1]
                                    nc.tensor.matmul(
                                        pp_k[:], w_sbs[1][:, dc, :],
                                        XT[:, dc, ts(st, 512)],
                                        start=(dc == 0), stop=(dc == DC - 1),
                                        skip_group_check=True)
                                    if dc == DC - 1:
                                        nc.scalar.copy(
                                            kt_sb[:, ts(st, 512)], pp_k[:])
                                else:
                                    _, j, dc = u
                                    sc = st * 4 + j
                                    pv_t = pp_v if j % 2 == 0 else pp_v2
                                    nc.tensor.matmul(
                                        pv_t[:, ts(j, 128)],
                                        XT[:, dc, ts(sc, 128)],
                                        w_sbs[2][:, dc, :],
                                        start=(dc == 0), stop=(dc == DC - 1),
                                        skip_group_check=True)
                                    if dc == DC - 1:
                                        nc.scalar.copy(
                                            v_sb[:, sc, :],
                                            pv_t[:, ts(j, 128)])

                        pot = psA.tile([128, 512], f32, tag="pot", name="pot")
                        eacc = pa.tile([128, 512], f32r, tag="eacc",
                                       name="eacc")

                        dist_c = int(9.0 / slope_c[h]) + 1

                        def attn_pre(kc, qt=qt, kc0=kc0, eacc=eacc,
                                     shift_sb=shift_sb, dist_c=dist_c):
                            # columns < r are fully masked; columns >= w are
                            # below the ALiBi decay threshold for every key
                            # in this block (bf16 matmuls run 1 cyc/row at
                            # any width)
                            r = max(0, 128 * kc - 512 * qt)
                            w = min(512, 128 * (kc + 1) + dist_c - 512 * qt)
                            pst = psST.tile([128, 512], f32, tag="pst",
                                            name="pst")
                            nc.tensor.matmul(pst[:, r:w],
                                             kt_sb[:, ts(kc, 128)],
                                             qt_sb[:, 512 * qt + r:
                                                   512 * qt + w],
                                             start=True, stop=True)
                            e_sb = pe_pool.tile([128, 512], bf16, tag="e",
                                                name="e_sb")
                            col = (h * DC + kc) * QT_TILES + qt
                            if steep:
                                t1 = pa.tile([128, 512], f32, tag="t1",
                                             name="t1")
                                nc.vector.scalar_tensor_tensor(
                                    t1[:, r:w], pst[:, r:w], SCALE,
                                    shift_sb[:, r:w],
                                    mybir.AluOpType.mult,
                                    mybir.AluOpType.add)
                                nc.scalar.activation(
                                    e_sb[:, r:w], t1[:, r:w], Exp,
                                    bias=alibi_sb[:, col:col + 1],
                                    scale=1.0)
                            else:
                                nc.scalar.activation(
                                    e_sb[:, r:w], pst[:, r:w], Exp,
                                    bias=alibi_sb[:, col:col + 1],
                                    scale=SCALE)
                            if kc >= 4 * qt:
                                # keep where qf - kp - r >= 0 (k <= q)
                                nc.gpsimd.affine_select(
                                    e_sb[:, r:r + 128],
                                    e_sb[:, r:r + 128],
                                    pattern=[[1, 128]],
                                    compare_op=mybir.AluOpType.is_ge,
                                    fill=0.0,
                                    base=0,
                                    channel_multiplier=-1)
                            # accumulate exp blocks for the softmax sums
                            if kc == kc0:
                                if w < 512:
                                    nc.vector.memset(
                                        eacc[:, w:].bitcast(f32), 0.0)
                                nc.vector.tensor_copy(eacc[:, r:w],
                                                      e_sb[:, r:w])
                            else:
                                nc.vector.tensor_add(eacc[:, r:w],
                                                     eacc[:, r:w],
                                                     e_sb[:, r:w])
                            return (r, w), e_sb

                        def attn_pv(kc, rw, e_sb, kc0=kc0, nkc=nkc, pot=pot,
                                    v_sb=v_sb):
                            r, w = rw
                            nc.tensor.matmul(pot[:, r:w], v_sb[:, kc, :],
                                             e_sb[:, r:w],
                                             start=(kc == kc0),
                                             stop=(kc == nkc - 1))

                        def attn_block(kc):
                            rw, e_sb = attn_pre(kc)
                            attn_pv(kc, rw, e_sb)

                        take_fillers(8)
                        # softmax sums + normalize of the PREVIOUS q-tile
                        # (here its DVE accumulation chain has fully drained)
                        emit_norm()
                        # blocks whose K/V are resident from earlier s-slices
                        for kc in range(kc0, 4 * qt):
                            attn_block(kc)
                            take_fillers(2)
                        take_fillers(DC + 4 * DC)   # drain the rest
                        # diagonal blocks (K/V of this s-slice just landed)
                        # run at the start of the NEXT stage, where the next
                        # Q projection hides their exp latency
                        pending_diag.append(
                            (attn_pre, attn_pv, list(range(4 * qt, nkc))))
                        pending.append((h, qt, pot, eacc))

                for h in range(NHG - 1, -1, -1):
                    emit_head(h)
                flush_diag()
                emit_norm()

            # ---- out^T = Wo_g^T @ O^T (XT pool closed) ----
            with (
                tc.tile_pool(name="otl", bufs=2) as pot_l,
                tc.tile_pool(name="ost", bufs=4) as post,
            ):
                wo_sb = wo_sb_box[0]
                for st in range(QT_TILES):
                    if st == 0:
                        ot_of = lambda h: ot0_tiles[h][:]
                    else:
                        ot_all = pot_l.tile([128, NHG, 512], bf16,
                                            tag="ot_all", name="ot_all")
                        for h in range(NHG):
                            nc.sync.dma_start(ot_all[:, h, :],
                                              ot_scratch[h, :, ts(st, 512)])
                        ot_of = lambda h, _t=ot_all: _t[:, h, :]
                    for mt in range(D // 128):
                        pp = psA.tile([128, 512], f32,
                                      tag="pp" if mt % 2 == 0 else "pot",
                                      bufs=3 if mt % 2 == 0 else None,
                                      name="pp")
                        for h in range(NHG):
                            nc.tensor.matmul(
                                pp[:],
                                wo_sb[:, h, ts(mt, 128)],
                                ot_of(h),
                                start=(h == 0), stop=(h == NHG - 1))
                        o_sb = post.tile([128, 512], f32, tag="osb",
                                         name="o_sb")
                        if st == QT_TILES - 1 and mt == D // 128 - 1:
                            for hf in range(2):
                                nc.scalar.copy(o_sb[:, ts(hf, 256)],
                                               pp[:, ts(hf, 256)])
                                nc.sync.dma_start(
                                    outT[ts(mt, 128),
                                         512 * st + 256 * hf:
                                         512 * st + 256 * (hf + 1)],
                                    o_sb[:, ts(hf, 256)])
                        else:
                            nc.scalar.copy(o_sb[:], pp[:])
                            nc.sync.dma_start(outT[ts(mt, 128), ts(st, 512)],
                                              o_sb[:])

    nc.compile()
    return nc


def _in_maps(x, Wq, Wk, Wv, Wo):
    import ml_dtypes

    bf16 = ml_dtypes.bfloat16
    slopes = np.asarray(_get_slopes(NH), dtype=np.float32)
    pos = np.arange(S, dtype=np.float32)
    dist = np.float32(S - 1) - pos                       # (S,)
    ones_col = np.ones((128, 1), np.float32)

    in_maps = []
    for b in range(B):
        xT = np.ascontiguousarray(x[b].T).astype(bf16)
        for g in range(2):
            heads = list(range(g, NH, 2))                 # interleaved
            sl = slopes[heads]                            # (8,)
            # alibi_b[p, ((h*DC+kc)*QT+qt)] = -sl[h]*dist[kc*128+p] + C[h,qt]
            ab = np.empty((128, NHG * DC * QT_TILES), np.float32)
            d2 = dist.reshape(DC, 128)                    # [kc, p]
            for h in range(NHG):
                for kc in range(DC):
                    a_col = (-sl[h] * d2[kc]).astype(np.float32)  # (128,)
                    for qt in range(QT_TILES):
                        if h < 2:
                            c = np.float32(0.0)
                        else:
                            q_mid = 512 * qt + 255.5
                            c = np.float32(sl[h] * (S - 1 - q_mid))
                        ab[:, (h * DC + kc) * QT_TILES + qt] = a_col + c
            alibi_q = (sl[:, None] * dist[None, :]).astype(np.float32)
            def w_fwd(W):
                # [D, NHG*HD] head-sliced -> [p, h, dc, f]
                arr = np.stack([W[:, h * HD:(h + 1) * HD] for h in heads],
                               axis=1)              # [D, NHG, HD]
                arr = arr.reshape(DC, 128, NHG, HD).transpose(1, 2, 0, 3)
                return np.ascontiguousarray(arr).astype(bf16)

            wo_arr = np.stack([Wo[h * HD:(h + 1) * HD, :] for h in heads],
                              axis=0)               # [NHG, HD, D]
            wo_arr = np.ascontiguousarray(
                wo_arr.transpose(1, 0, 2)).astype(bf16)   # [p, NHG, D]
            in_maps.append({
                "x": xT,
                "wq": w_fwd(Wq),
                "wk": w_fwd(Wk),
                "wv": w_fwd(Wv),
                "wo": wo_arr,
                "alibi_b": ab,
                "alibi_q": alibi_q,
                "ones_col": ones_col,
            })
    return in_maps


def kernel(x, Wq, Wk, Wv, Wo, _trace=False):
    from concourse.bass_utils import run_bass_kernel_spmd

    if "nc" not in _cache:
        _cache["nc"] = _build()
    nc = _cache["nc"]

    res = run_bass_kernel_spmd(
        nc, _in_maps(x, Wq, Wk, Wv, Wo), core_ids=list(range(2 * B)),
        trace=_trace)
    _cache["last_exec_time_ns"] = res.exec_time_ns

    out = np.empty((B, S, D), dtype=np.float32)
    for b in range(B):
        out[b] = (res.results[2 * b]["outT"] + res.results[2 * b + 1]["outT"]).T
    return out


# revision 30
# speedup vs baseline: 1.0021x; 1.0016x over previous
"""Causal attention with ALiBi for Trainium2, tensor-parallel over heads x
data-parallel over batch (8 NeuronCores).

Problem: B=4, S=2048, D=2048, NH=16, HD=128, fp32.
  q/k/v = x @ Wq/Wk/Wv ; scores = q k^T / sqrt(HD) + alibi ; causal softmax ;
  out = (probs @ v) @ Wo

Sharding: core (b, j) handles batch b and the 8 interleaved heads
  j, j+2, ..., j+14 (interleaving balances steep/shallow ALiBi slopes so the
  per-core block-skipping is symmetric).  Each core returns out_partial^T;
  the host sums the two per-batch partials and transposes back.

On-core pipeline (bf16 matmul operands everywhere, fp32 PSUM accumulation;
all inputs arrive host-pre-arranged so every DMA reads >=1KB-contiguous runs
per partition: x as x^T bf16 [128, dc, s], W's as [p, head, dc, f]).
Heads are processed shallowest-slope first so the attention-heavy heads run
while x is still streaming in; the two steep heads drain the tail.

Per head, a software-pipelined schedule per s-slice st (= q-tile qt):
  1. deferred diagonal blocks of q-tile st-1: their scores/shift/exp/mask
     chains are emitted first, and their PV matmuls land right after the Q
     projection matmuls below, by which time the chains have drained.
  2. Q^T[:, st] = Wq-chunks^T @ XT  (16 matmuls, PSUM, ACT copy to bf16)
  3. softmax sums + normalize of q-tile st-1 (one ones-column matmul turns
     the elementwise-accumulated exp tile into the softmax sums; reciprocal
     on DVE, gpsimd partition_broadcast, normalize-multiply on DVE).  One
     stage late so the PE never waits on the DVE accumulation chain.
  4. attention blocks of q-tile st whose K/V are already resident
     (kc < 4st), with the K^T and V matmuls of THIS s-slice interleaved
     between blocks as PE filler work, so the PE never waits on the ACT exp
     chain.  V is computed directly in [k, hd] layout with XT chunks as the
     stationary operand (no PE transposes anywhere in the kernel); its four
     PSUM accumulation groups alternate between two banks so a group's
     start never waits on the previous group's PSUM->SBUF copy.
  5. the 4 diagonal blocks' scores/exp chains (their K/V just landed); a
     gpsimd affine_select masks the partial causal band; their PVs defer
     to step 1 of the next stage.
Softmax: exp(scores*scale + alibi[k] + shift[q]); the per-q shift folds
into the ACT bias column per (head, q-tile) for shallow heads (exact
cancellation) and is applied per-block on DVE for the 2 steep heads.
Blocks whose ALiBi decay is < e^-9 of the softmax sum are skipped, and
within kept blocks the q-columns beyond the decay window are not computed
(bf16 matmuls run 1 cyc/row at any width, so narrow blocks are cheap).
O^T tiles for qt=0 stay in SBUF (bf16); qt>0 spill to DRAM per (h, q-tile).
out^T = Wo_j^T @ O^T accumulated over the 8 heads (bf16); Wo prefetches as
one contiguous tile during the last head so the output stage starts with no
bubble, and its PSUM tiles alternate between two rings so the output copies
are never on the critical path.
"""

import math

import numpy as np

B, S, D, NH = 4, 2048, 2048, 16
HD = D // NH            # 128
NHG = NH // 2           # heads per core
DC = D // 128           # 16 d-chunks
QT_TILES = S // 512     # 4 q tiles
SCALE = 1.0 / math.sqrt(HD)

_cache = {}


def _get_slopes(n):
    def pow2(n):
        start = 2 ** (-(2 ** (-(math.log2(n) - 3))))
        return [start * start**i for i in range(n)]

    if math.log2(n).is_integer():
        return pow2(n)
    c = 2 ** math.floor(math.log2(n))
    return pow2(c) + _get_slopes(2 * c)[0::2][: n - c]


def _build():
    import concourse.bacc as bacc
    import concourse.mybir as mybir
    import concourse.tile as tile
    from concourse.bass import ts

    f32 = mybir.dt.float32
    f32r = mybir.dt.float32r
    bf16 = mybir.dt.bfloat16
    Exp = mybir.ActivationFunctionType.Exp

    nc = bacc.Bacc()
    # x arrives pre-transposed (host-side) as bf16 [D, S]
    x_in = nc.declare_dram_parameter("x", [D, S], bf16, isOutput=False)
    # weights arrive pre-arranged by the host as [p, head, dc, f] so each
    # per-head DMA reads 4KB-contiguous runs per partition
    wq_in = nc.declare_dram_parameter("wq", [128, NHG, DC, HD], bf16,
                                      isOutput=False)
    wk_in = nc.declare_dram_parameter("wk", [128, NHG, DC, HD], bf16,
                                      isOutput=False)
    wv_in = nc.declare_dram_parameter("wv", [128, NHG, DC, HD], bf16,
                                      isOutput=False)
    wo_in = nc.declare_dram_parameter("wo", [128, NHG, D], bf16,
                                      isOutput=False)
    # alibi_b[p, ((h*16+kc)*4+qt)] = -slope_h*(S-1-(kc*128+p)) + C[h,qt]
    # C folds the per-q-tile softmax shift for heads with small slope.
    alibi_b_in = nc.declare_dram_parameter(
        "alibi_b", [128, NHG * DC * QT_TILES], f32, isOutput=False)
    # alibi_q[h, q] = +slope_h * (S-1 - q)   (per-query shift)
    alibi_q_in = nc.declare_dram_parameter("alibi_q", [NHG, S], f32,
                                           isOutput=False)
    ones_col_in = nc.declare_dram_parameter("ones_col", [128, 1], f32r,
                                            isOutput=False)
    outT = nc.declare_dram_parameter("outT", [D, S], f32, isOutput=True)

    ot_scratch = nc.dram_tensor("ot_scratch", [NHG, 128, S], bf16)

    # heads are interleaved across the two cores of a batch (core parity j
    # gets global heads j, j+2, ...).  Skip counts use the SHALLOWER
    # parity's slope so one SPMD program is valid for both.
    slope_c = [0.7071067811865476 ** (2 * hh + 2) for hh in range(NHG)]

    def n_skip(h, qt):
        # contribution of a skipped block is < e^-9 of the softmax sum
        dist = int(9.0 / slope_c[h]) + 1
        return max(0, (512 * qt - dist - 127) // 128 + 1)

    with tile.TileContext(nc) as tc:
        with (
            tc.tile_pool(name="consts", bufs=1) as pc,
            tc.tile_pool(name="oz", bufs=8) as po0,
            tc.tile_pool(name="wohi", bufs=1) as pwo_hi,
            tc.tile_pool(name="psA", bufs=2, space="PSUM") as psA,
            tc.tile_pool(name="psST", bufs=3, space="PSUM") as psST,
        ):
            alibi_sb = pc.tile([128, NHG * DC * QT_TILES], f32,
                               name="alibi_sb")
            ones_col = pc.tile([128, 1], f32r, name="ones_col_sb")

            ot0_tiles = {}       # per-head qt=0 O^T tiles, kept in SBUF
            wo_sb_box = []

            def load_wo():
                wo_sb = pwo_hi.tile([128, NHG, D], bf16, tag="wo",
                                    name="wo_sb")
                nc.sync.dma_start(wo_sb[:], wo_in[:])
                wo_sb_box.append(wo_sb)

            with (
                tc.tile_pool(name="xt", bufs=1) as pxt,
                tc.tile_pool(name="wp", bufs=2) as pw,
                tc.tile_pool(name="qkv2", bufs=2) as pq2,
                tc.tile_pool(name="qkv", bufs=2) as pq,
                tc.tile_pool(name="att", bufs=2) as pa,
                tc.tile_pool(name="epool", bufs=8) as pe_pool,
                tc.tile_pool(name="small", bufs=2) as psm,
            ):
                XT = pxt.tile([128, DC, S], bf16, name="XT")
                xt_view = x_in.rearrange("(dc p) s -> p dc s", p=128)

                # pending softmax-sum/normalize work, emitted one stage late:
                # (h, qt, pot, eacc)
                pending = []
                # pending diagonal attention blocks: (attn_block_fn, kcs)
                pending_diag = []

                def flush_diag():
                    if not pending_diag:
                        return
                    pre, pv, kcs = pending_diag.pop()
                    done = [(kc,) + pre(kc) for kc in kcs]
                    for kc, rw, e_sb in done:
                        pv(kc, rw, e_sb)

                def emit_norm():
                    if not pending:
                        return
                    h, qt, pot, eacc = pending.pop()
                    psums = psA.tile([1, 512], f32, tag="pot", name="psums")
                    nc.tensor.matmul(psums[:], ones_col[:], eacc[:],
                                     start=True, stop=True)
                    recip = psm.tile([1, 512], f32, tag="recip", name="recip")
                    nc.vector.reciprocal(recip[:], psums[:])
                    bc_sb = pa.tile([128, 512], f32, tag="bc", name="bc_sb")
                    nc.gpsimd.partition_broadcast(bc_sb[:], recip[:])
                    if qt == 0:
                        ot_sb = po0.tile([128, 512], bf16, tag="ot0",
                                         name="ot0_sb")
                        ot0_tiles[h] = ot_sb
                    else:
                        ot_sb = pa.tile([128, 512], bf16, tag="ot",
                                        name="ot_sb")
                    nc.vector.tensor_mul(out=ot_sb[:], in0=pot[:],
                                         in1=bc_sb[:])
                    if qt != 0:
                        nc.sync.dma_start(ot_scratch[h, :, ts(qt, 512)],
                                          ot_sb[:])

                def emit_head(h):
                    qt_sb = pq2.tile([128, S], bf16, tag="QT", name="qt_sb")
                    kt_sb = pq2.tile([128, S], bf16, tag="KT", name="kt_sb")
                    v_sb = pq.tile([128, DC, HD], bf16, tag="V", name="v_sb")
                    w_sbs = []
                    for w_in, wtag in ((wq_in, "wq"), (wk_in, "wk"),
                                       (wv_in, "wv")):
                        w_sb = pw.tile([128, DC, HD], bf16, tag=wtag,
                                       name="w_sb")
                        if h == NHG - 1:
                            # first head: first x quarter, then wq, then the
                            # remaining quarters; wk/wv follow (needed only
                            # once the K/V fillers start)
                            if wtag == "wq":
                                nc.sync.dma_start(
                                    XT[:, ts(0, 2), ts(0, 512)],
                                    xt_view[:, ts(0, 2), ts(0, 512)])
                                nc.scalar.dma_start(w_sb[:], w_in[:, h])
                                nc.sync.dma_start(
                                    XT[:, ts(1, 2), ts(0, 512)],
                                    xt_view[:, ts(1, 2), ts(0, 512)])
                                for qi in range(1, 4):
                                    nc.sync.dma_start(
                                        XT[:, ts(qi, 4), ts(0, 512)],
                                        xt_view[:, ts(qi, 4), ts(0, 512)])
                            elif wtag == "wk":
                                nc.sync.dma_start(alibi_sb[:], alibi_b_in[:])
                                nc.sync.dma_start(ones_col[:],
                                                  ones_col_in[:])
                                nc.scalar.dma_start(w_sb[:], w_in[:, h])
                            else:
                                nc.scalar.dma_start(w_sb[:], w_in[:, h])
                        else:
                            nc.sync.dma_start(w_sb[:], w_in[:, h])
                        w_sbs.append(w_sb)
                    if h == NHG - 1:
                        for st in range(1, QT_TILES):
                            nc.sync.dma_start(XT[:, :, ts(st, 512)],
                                              xt_view[:, :, ts(st, 512)])
                    if h == 0:
                        load_wo()

                    steep = h < 2
                    for st in range(QT_TILES):
                        qt = st
                        nkc = 4 * (qt + 1)
                        kc0 = n_skip(h, qt)
                        shift_sb = None
                        if steep:
                            shift_sb = psm.tile([128, 512], f32, tag="shift",
                                                name="shift_sb")
                            nc.sync.dma_start(
                                shift_sb[:],
                                alibi_q_in[h, ts(qt, 512)]
                                .partition_broadcast(128))

                        # previous q-tile's diagonal blocks: emit their
                        # scores/exp chains first, then interleave their PV
                        # matmuls into the Q projection so the PE never waits
                        # on the 4-hop scores->shift->exp->mask latency
                        diag_pvs = []
                        if pending_diag:
                            pre, pv, kcs = pending_diag.pop()
                            for kc in kcs:
                                diag_pvs.append((pv, kc) + pre(kc))

                        # ---- Q projection for this s-slice ----
                        pp_q = psA.tile([128, 512], f32, tag="pp", bufs=3, name="pp_q")
                        for dc in range(DC):
                            nc.tensor.matmul(
                                pp_q[:], w_sbs[0][:, dc, :],
                                XT[:, dc, ts(st, 512)],
                                start=(dc == 0), stop=(dc == DC - 1),
                                skip_group_check=True)
                        for pv, kc, rw, e_sb in diag_pvs:
                            pv(kc, rw, e_sb)
                        nc.scalar.copy(qt_sb[:, ts(st, 512)], pp_q[:])

                        # ---- filler stream: K^T and V matmuls of this
                        # s-slice, interleaved between attention blocks so
                        # the PE never drains while ACT runs the exp chain.
                        pp_k = psA.tile([128, 512], f32, tag="pp", bufs=3, name="pp_k")
                        pp_v = psA.tile([128, 512], f32, tag="pp", bufs=3, name="pp_v")
                        pp_v2 = psA.tile([128, 512], f32, tag="pp", bufs=3,
                                         name="pp_v2")

                        def fill_units():
                            for dc in range(DC):
                                yield ("K", dc)
                            for j in range(4):
                                for dc in range(DC):
                                    yield ("V", j, dc)

                        filler = fill_units()

                        def take_fillers(n):
                            for _ in range(n):
                                u = next(filler, None)
                                if u is None:
                                    return
                                if u[0] == "K":
                                    dc = u[1]
                                    nc.tensor.matmul(
                                        pp_k[:], w_sbs[1][:, dc, :],
                                        XT[:, dc, ts(st, 512)],
                                        start=(dc == 0), stop=(dc == DC - 1),
                                        skip_group_check=True)
                                    if dc == DC - 1:
                                        nc.scalar.copy(
                                            kt_sb[:, ts(st, 512)], pp_k[:])
                                else:
                                    _, j, dc = u
                                    sc = st * 4 + j
                                    pv_t = pp_v if j % 2 == 0 else pp_v2
                                    nc.tensor.matmul(
                                        pv_t[:, ts(j, 128)],
                                        XT[:, dc, ts(sc, 128)],
                                        w_sbs[2][:, dc, :],
                                        start=(dc == 0), stop=(dc == DC - 1),
                                        skip_group_check=True)
                                    if dc == DC - 1:
                                        nc.vector.tensor_copy(
                                            v_sb[:, sc, :],
                                            pv_t[:, ts(j, 128)])

                        pot = psA.tile([128, 512], f32, tag="pot", name="pot")
                        eacc = pa.tile([128, 512], f32r, tag="eacc",
                                       name="eacc")

                        dist_c = int(9.0 / slope_c[h]) + 1

                        def attn_pre(kc, qt=qt, kc0=kc0, eacc=eacc,
                                     shift_sb=shift_sb, dist_c=dist_c):
                            # columns < r are fully masked; columns >= w are
                            # below the ALiBi decay threshold for every key
                            # in this block (bf16 matmuls run 1 cyc/row at
                            # any width)
                            r = max(0, 128 * kc - 512 * qt)
                            w = min(512, 128 * (kc + 1) + dist_c - 512 * qt)
                            pst = psST.tile([128, 512], f32, tag="pst",
                                            name="pst")
                            nc.tensor.matmul(pst[:, r:w],
                                             kt_sb[:, ts(kc, 128)],
                                             qt_sb[:, 512 * qt + r:
                                                   512 * qt + w],
                                             start=True, stop=True)
                            e_sb = pe_pool.tile([128, 512], bf16, tag="e",
                                                name="e_sb")
                            col = (h * DC + kc) * QT_TILES + qt
                            if steep:
                                t1 = pa.tile([128, 512], f32, tag="t1",
                                             name="t1")
                                nc.vector.scalar_tensor_tensor(
                                    t1[:, r:w], pst[:, r:w], SCALE,
                                    shift_sb[:, r:w],
                                    mybir.AluOpType.mult,
                                    mybir.AluOpType.add)
                                nc.scalar.activation(
                                    e_sb[:, r:w], t1[:, r:w], Exp,
                                    bias=alibi_sb[:, col:col + 1],
                                    scale=1.0)
                            else:
                                nc.scalar.activation(
                                    e_sb[:, r:w], pst[:, r:w], Exp,
                                    bias=alibi_sb[:, col:col + 1],
                                    scale=SCALE)
                            if kc >= 4 * qt:
                                # keep where qf - kp - r >= 0 (k <= q)
                                nc.gpsimd.affine_select(
                                    e_sb[:, r:r + 128],
                                    e_sb[:, r:r + 128],
                                    pattern=[[1, 128]],
                                    compare_op=mybir.AluOpType.is_ge,
                                    fill=0.0,
                                    base=0,
                                    channel_multiplier=-1)
                            # accumulate exp blocks for the softmax sums
                            if kc == kc0:
                                if w < 512:
                                    nc.vector.memset(
                                        eacc[:, w:].bitcast(f32), 0.0)
                                nc.vector.tensor_copy(eacc[:, r:w],
                                                      e_sb[:, r:w])
                            else:
                                nc.vector.tensor_add(eacc[:, r:w],
                                                     eacc[:, r:w],
                                                     e_sb[:, r:w])
                            return (r, w), e_sb

                        def attn_pv(kc, rw, e_sb, kc0=kc0, nkc=nkc, pot=pot,
                                    v_sb=v_sb):
                            r, w = rw
                            nc.tensor.matmul(pot[:, r:w], v_sb[:, kc, :],
                                             e_sb[:, r:w],
                                             start=(kc == kc0),
                                             stop=(kc == nkc - 1))

                        def attn_block(kc):
                            rw, e_sb = attn_pre(kc)
                            attn_pv(kc, rw, e_sb)

                        take_fillers(8)
                        # softmax sums + normalize of the PREVIOUS q-tile
                        # (here its DVE accumulation chain has fully drained)
                        emit_norm()
                        # blocks whose K/V are resident from earlier s-slices
                        for kc in range(kc0, 4 * qt):
                            attn_block(kc)
                            take_fillers(2)
                        take_fillers(DC + 4 * DC)   # drain the rest
                        # diagonal blocks (K/V of this s-slice just landed)
                        # run at the start of the NEXT stage, where the next
                        # Q projection hides their exp latency
                        pending_diag.append(
                            (attn_pre, attn_pv, list(range(4 * qt, nkc))))
                        pending.append((h, qt, pot, eacc))

                for h in range(NHG - 1, -1, -1):
                    emit_head(h)
                flush_diag()
                emit_norm()

            # ---- out^T = Wo_g^T @ O^T (XT pool closed) ----
            with (
                tc.tile_pool(name="otl", bufs=2) as pot_l,
                tc.tile_pool(name="ost", bufs=4) as post,
            ):
                wo_sb = wo_sb_box[0]
                for st in range(QT_TILES):
                    if st == 0:
                        ot_of = lambda h: ot0_tiles[h][:]
                    else:
                        ot_all = pot_l.tile([128, NHG, 512], bf16,
                                            tag="ot_all", name="ot_all")
                        for h in range(NHG):
                            nc.sync.dma_start(ot_all[:, h, :],
                                              ot_scratch[h, :, ts(st, 512)])
                        ot_of = lambda h, _t=ot_all: _t[:, h, :]
                    for mt in range(D // 128):
                        pp = psA.tile([128, 512], f32,
                                      tag="pp" if mt % 2 == 0 else "pot",
                                      bufs=3 if mt % 2 == 0 else None,
                                      name="pp")
                        for h in range(NHG):
                            nc.tensor.matmul(
                                pp[:],
                                wo_sb[:, h, ts(mt, 128)],
                                ot_of(h),
                                start=(h == 0), stop=(h == NHG - 1))
                        o_sb = post.tile([128, 512], f32, tag="osb",
                                         name="o_sb")
                        if st == QT_TILES - 1 and mt == D // 128 - 1:
                            for hf in range(2):
                                nc.scalar.copy(o_sb[:, ts(hf, 256)],
                                               pp[:, ts(hf, 256)])
                                nc.sync.dma_start(
                                    outT[ts(mt, 128),
                                         512 * st + 256 * hf:
                                         512 * st + 256 * (hf + 1)],
                                    o_sb[:, ts(hf, 256)])
                        else:
                            nc.scalar.copy(o_sb[:], pp[:])
                            nc.sync.dma_start(outT[ts(mt, 128), ts(st, 512)],
                                              o_sb[:])

    nc.compile()
    return nc


def _in_maps(x, Wq, Wk, Wv, Wo):
    import ml_dtypes

    bf16 = ml_dtypes.bfloat16
    slopes = np.asarray(_get_slopes(NH), dtype=np.float32)
    pos = np.arange(S, dtype=np.float32)
    dist = np.float32(S - 1) - pos                       # (S,)
    ones_col = np.ones((128, 1), np.float32)

    in_maps = []
    for b in range(B):
        xT = np.ascontiguousarray(x[b].T).astype(bf16)
        for g in range(2):
            heads = list(range(g, NH, 2))                 # interleaved
            sl = slopes[heads]                            # (8,)
            # alibi_b[p, ((h*DC+kc)*QT+qt)] = -sl[h]*dist[kc*128+p] + C[h,qt]
            ab = np.empty((128, NHG * DC * QT_TILES), np.float32)
            d2 = dist.reshape(DC, 128)                    # [kc, p]
            for h in range(NHG):
                for kc in range(DC):
                    a_col = (-sl[h] * d2[kc]).astype(np.float32)  # (128,)
                    for qt in range(QT_TILES):
                        if h < 2:
                            c = np.float32(0.0)
                        else:
                            q_mid = 512 * qt + 255.5
                            c = np.float32(sl[h] * (S - 1 - q_mid))
                        ab[:, (h * DC + kc) * QT_TILES + qt] = a_col + c
            alibi_q = (sl[:, None] * dist[None, :]).astype(np.float32)
            def w_fwd(W):
                # [D, NHG*HD] head-sliced -> [p, h, dc, f]
                arr = np.stack([W[:, h * HD:(h + 1) * HD] for h in heads],
                               axis=1)              # [D, NHG, HD]
                arr = arr.reshape(DC, 128, NHG, HD).transpose(1, 2, 0, 3)
                return np.ascontiguousarray(arr).astype(bf16)

            wo_arr = np.stack([Wo[h * HD:(h + 1) * HD, :] for h in heads],
                              axis=0)               # [NHG, HD, D]
            wo_arr = np.ascontiguousarray(
                wo_arr.transpose(1, 0, 2)).astype(bf16)   # [p, NHG, D]
            in_maps.append({
                "x": xT,
                "wq": w_fwd(Wq),
                "wk": w_fwd(Wk),
                "wv": w_fwd(Wv),
                "wo": wo_arr,
                "alibi_b": ab,
                "alibi_q": alibi_q,
                "ones_col": ones_col,
            })
    return in_maps


def kernel(x, Wq, Wk, Wv, Wo, _trace=False):
    from concourse.bass_utils import run_bass_kernel_spmd

    if "nc" not in _cache:
        _cache["nc"] = _build()
    nc = _cache["nc"]

    res = run_bass_kernel_spmd(
        nc, _in_maps(x, Wq, Wk, Wv, Wo), core_ids=list(range(2 * B)),
        trace=_trace)
    _cache["last_exec_time_ns"] = res.exec_time_ns

    out = np.empty((B, S, D), dtype=np.float32)
    for b in range(B):
        out[b] = (res.results[2 * b]["outT"] + res.results[2 * b + 1]["outT"]).T
    return out
